# revision 22
# baseline (speedup 1.0000x reference)
"""Trainium2 Bass kernel for nn_MultiHeadAttention_34162169872901.

MultiHeadAttention (B=4, S=2048, d_model=512, 8 heads, d_k=64) with a
relative-position bias table (511 entries, clamp +-255) and an all-ones mask.

Sharding (8 NeuronCores): core c handles batch b = c//2 and 4 of the 8 heads
(c%2 selects the head half) -- data parallel on B, tensor parallel on heads.
Each core computes its 4 heads' Q/K/V projections, the full attention for its
batch, normalization, and its partial output projection; the host sums the two
partial outputs per batch (and adds the output bias bo).

On-device layout / algorithm per core:
  - Host pre-transposes activations to [d_model, S] so the contraction dim is
    on partitions, and pre-arranges weight slices; the 1/sqrt(d_k) scale is
    folded into Wq.
  - Projections produce QT/KT in [head-pair(128), S] layout and V in natural
    [S, d] layout (with a ones column per head for the softmax denominator).
  - Scores are computed transposed (S^T[k, q], k on partitions) so the
    relative-position bias becomes per-(k-tile, q-chunk) Toeplitz blocks;
    blocks fully outside the clamp band are constants folded into the exp's
    per-partition bias; in-band blocks multiply exp(scores) by precomputed
    exp(bias) blocks in bf16 on the vector engine.
  - AV matmul uses V (plus a ones column) as the stationary operand, giving
    ctx^T[d, q] and the softmax denominator l[q] in one accumulation chain.
  - Normalization: approximate reciprocal of l, broadcast to 64 rows via a
    tiny matmul, fused multiply into the O-projection stationary.
  - O-projection accumulates all 4 local heads into [q, 512] PSUM tiles.
"""

import sys
import types

import numpy as np

B = 4
S = 2048
D = 512
NHEAD = 8
DK = 64
NCORES = 8
MAX_REL = 255
NKT = S // 128   # 16 k-tiles
NU = S // 512    # 4 q-units


def _install_axon_hooks():
    """Provide antenv.axon_hooks (missing in this image) so bass_utils'
    trace path can be used; harmless when tracing is off."""
    try:
        import antenv
    except ImportError:
        return
    try:
        from antenv.axon_hooks import get_axon_ntff_profile_hook  # noqa: F401
        return
    except ImportError:
        pass
    hook = None
    try:
        from trn_agent_boot.trn_boot import _ntff_profile_via_ctypes
        hook = _ntff_profile_via_ctypes("/opt/axon/libaxon_pjrt.so")
    except Exception:
        hook = None
    m = types.ModuleType("antenv.axon_hooks")
    m.get_axon_ntff_profile_hook = lambda: hook
    m.set_axon_ntff_profile_hook = lambda h: None
    sys.modules["antenv.axon_hooks"] = m
    antenv.axon_hooks = m


_install_axon_hooks()

import concourse.bass as bass  # noqa: E402
import concourse.bacc as bacc  # noqa: E402
import concourse.mybir as mybir  # noqa: E402
from concourse import tile  # noqa: E402
from concourse.bass_utils import run_bass_kernel_spmd  # noqa: E402
from concourse.vector_clock import ScopedClock as _ScopedClock  # noqa: E402

f32 = mybir.dt.float32
f32r = mybir.dt.float32r
bf16 = mybir.dt.bfloat16
f16 = mybir.dt.float16
AF = mybir.ActivationFunctionType


def _patched_drain_and_barrier(self, tick_clock, wait_clock):
    # walrus in this container rejects >2 sem waits on one instruction; emit
    # the tail-drain waits as standalone wait instructions instead.
    nc = self.nc
    dummy = mybir.InstNoOp(name="drain-wait-probe", engine=mybir.EngineType.SP)
    wait_clock.add_sem_waits(dummy, _ScopedClock({None: tick_clock.global_clock}))
    handles = {h.name: h for h in self.sems.allocated().values()}
    si = dummy.sync_info
    for w in (si.on_wait if si is not None else []):
        nc.sync.wait_ge(handles[w.ant_name], w.wait_value)
    nc.sync.drain()
    nc.all_engine_barrier()
    popped = nc._tile_sem_poison_stack.pop()
    assert popped is self._sem_poison
    nc.clear_and_free_semaphores(list(self.sems.allocated().values()))
    nc.all_engine_barrier()


tile.TileContext._drain_and_barrier = _patched_drain_and_barrier


def _delta(t, u):
    # key-tile offset minus query-chunk offset; bias entry index is
    # delta + (p - f) + 255 clipped to [0, 510]
    return 128 * t - 512 * u


def _cls(t, u):
    d = _delta(t, u)
    if d <= -384:
        return 1  # whole block clamps to table[0]
    if d >= 768:
        return 2  # whole block clamps to table[510]
    return 0      # in-band: needs the Toeplitz block


def _didx(t, u):
    return (_delta(t, u) + 256) // 128  # 0..7 for in-band blocks


def build_program():
    nc = bacc.Bacc()

    xqT = nc.declare_dram_parameter("xqT", [D, S], f16, isOutput=False)
    xkT = nc.declare_dram_parameter("xkT", [D, S], f16, isOutput=False)
    xvT = nc.declare_dram_parameter("xvT", [D, S], f16, isOutput=False)
    wq = nc.declare_dram_parameter("wq", [128, 4, 256], f16, isOutput=False)
    wk = nc.declare_dram_parameter("wk", [128, 4, 256], f16, isOutput=False)
    wv = nc.declare_dram_parameter("wv", [128, 4, 256], f16, isOutput=False)
    wo = nc.declare_dram_parameter("wo", [64, 4, 512], f32r, isOutput=False)
    ebd = nc.declare_dram_parameter("eb", [128, 4, 8, 512], f16, isOutput=False)
    cbd = nc.declare_dram_parameter("cb", [128, 4, 3], f32, isOutput=False)
    outd = nc.declare_dram_parameter("out", [S, D], f32, isOutput=True)

    with tile.TileContext(nc) as tc:
        with (
            tc.tile_pool(name="sb", bufs=1) as pool,
            tc.tile_pool(name="xt", bufs=2) as xpool,
            tc.tile_pool(name="pt", bufs=4) as ppool,
            tc.tile_pool(name="cxp", bufs=3) as cpool,
            tc.tile_pool(name="obp", bufs=4) as opool,
        ):
            # ---- persistent SBUF tiles -------------------------------------
            wq_sb = pool.tile([128, 4, 256], f16, tag="wq")
            wk_sb = pool.tile([128, 4, 256], f16, tag="wk")
            wv_sb = pool.tile([128, 4, 256], f16, tag="wv")
            wo_sb = pool.tile([64, 4, 512], f32r, tag="wo")
            eb_sb = pool.tile([128, 4, 8, 512], f16, tag="eb")
            cb_sb = pool.tile([128, 4, 3], f32, tag="cb")
            qt_sb = pool.tile([128, 2, S], f16, tag="qt")
            kt_sb = pool.tile([128, 2, S], f16, tag="kt")
            v_sb = pool.tile([128, NKT, 4 * 65], f16, tag="v")
            ones_c = pool.tile([1, 64], mybir.dt.float16, tag="ones")
            warm = pool.tile([128, 16], f32, tag="warm")

            nc.sync.dma_start(wq_sb[:], wq[:])
            nc.sync.dma_start(wk_sb[:], wk[:])
            nc.sync.dma_start(wv_sb[:], wv[:])
            nc.vector.memset(ones_c[:], 1.0)
            # preload the exp table while DMAs stream in
            nc.vector.memset(warm[:], 0.0)
            nc.scalar.activation(warm[:], warm[:], AF.Exp, bias=0.0, scale=1.0)

            # ---- phase A: projections --------------------------------------
            with tc.tile_pool(name="pa", bufs=8, space="PSUM") as pa:
                _dma_rest = True
                # Q and K -> [head-pair 128, S] (transposed) layout
                # xv streams on the gpsimd queue ahead of eb/cb/wo
                xts = []
                for ct in range(4):
                    xv_t = xpool.tile([128, S], f16, tag="xv", bufs=4, name=f"xv{ct}")
                    nc.gpsimd.dma_start(xv_t[:], xvT[ct * 128:(ct + 1) * 128, :])
                    xts.append(xv_t)
                nc.gpsimd.dma_start(eb_sb[:], ebd[:])
                nc.gpsimd.dma_start(cb_sb[:], cbd[:])
                nc.gpsimd.dma_start(wo_sb[:], wo[:])
                # K and Q hp0 stream per-arriving c-tile (8 banks); hp1 and V
                # run as dense backlog to keep the PE warm into phase B
                kts, qts = [], []
                for ct in range(4):
                    xk_t = xpool.tile([128, S], f16, tag="xt", name=f"xk{ct}")
                    nc.sync.dma_start(xk_t[:], xkT[ct * 128:(ct + 1) * 128, :])
                    kts.append(xk_t)
                    xq_t = xpool.tile([128, S], f16, tag="xq", bufs=4, name=f"xq{ct}")
                    nc.scalar.dma_start(xq_t[:], xqT[ct * 128:(ct + 1) * 128, :])
                    qts.append(xq_t)
                for hp in range(2):
                    pks = {sc: pa.tile([128, 512], f32, tag="pa", name=f"pk{hp}_{sc}")
                           for sc in range(4)}
                    pqs = {sc: pa.tile([128, 512], f32, tag="pa", name=f"pq{hp}_{sc}")
                           for sc in range(4)}
                    for ct in range(4):
                        for sc in range(4):
                            nc.tensor.matmul(
                                pks[sc][:],
                                lhsT=wk_sb[:, ct, hp * 128:(hp + 1) * 128],
                                rhs=kts[ct][:, sc * 512:(sc + 1) * 512],
                                start=(ct == 0), stop=(ct == 3),
                            )
                            nc.tensor.matmul(
                                pqs[sc][:],
                                lhsT=wq_sb[:, ct, hp * 128:(hp + 1) * 128],
                                rhs=qts[ct][:, sc * 512:(sc + 1) * 512],
                                start=(ct == 0), stop=(ct == 3),
                            )
                    for sc in range(4):
                        nc.vector.tensor_copy(
                            kt_sb[:, hp, sc * 512:(sc + 1) * 512], pks[sc][:])
                        nc.vector.tensor_copy(
                            qt_sb[:, hp, sc * 512:(sc + 1) * 512], pqs[sc][:])
                # V -> natural [s, d] layout in bf16, one 65-col group per head
                # one accumulation group open per bank at a time: run each
                # s-tile's 4-step contraction to completion (xv tiles resident)
                pv = [pa.tile([128, 512], f32, tag="pa", name=f"pv{i}") for i in range(8)]
                for st in range(NKT):
                    for ct in range(4):
                        nc.tensor.matmul(
                            pv[st // 2][:, (st % 2) * 256:(st % 2) * 256 + 256],
                            lhsT=xts[ct][:, st * 128:(st + 1) * 128],
                            rhs=wv_sb[:, ct, :],
                            start=(ct == 0), stop=(ct == 3),
                        )
                for st in range(NKT):
                    vslice = v_sb[:, st, :].rearrange("p (h x) -> p h x", x=65)
                    nc.vector.tensor_copy(
                        vslice[:, :, 0:64],
                        pv[st // 2][:, (st % 2) * 256:(st % 2) * 256 + 256]
                        .rearrange("p (h x) -> p h x", x=64),
                    )
                    nc.vector.memset(vslice[:, :, 64:65], 1.0)

            # ---- phase B: attention + normalization + O-projection --------
            import os as _os
            _phase = _os.environ.get("KPHASE", "full")
            if _phase == "A":
                ob0 = opool.tile([128, 512], f32, tag="ob", name="ob0")
                nc.vector.tensor_copy(ob0[:], qt_sb[:, 0, 0:512].bitcast(f32))
                nc.sync.dma_start(outd[0:128, :], ob0[:])
                ob1 = opool.tile([128, 256], f32, tag="ob1", name="ob1")
                nc.vector.tensor_copy(ob1[:], v_sb[:, 0, 0:256])
                nc.sync.dma_start(outd[128:256, 0:256], ob1[:])
            _enable_b = _phase != "A"
            with (
                tc.tile_pool(name="sc", bufs=3, space="PSUM") as scp,
                tc.tile_pool(name="c1", bufs=1, space="PSUM") as c1p,
            ):
                _lvl = int(_os.environ.get("KLEVEL", "5"))
                for u in (range(NU) if _enable_b else []):
                    cx = {}
                    for hp in range(2):
                        if _lvl >= 3:
                            ctxp = [c1p.tile([65, 512], f32, tag="cp", bufs=2, name=f"ctxp{i}") for i in range(2)]
                        nav = [0, 0]
                        for g in range(NKT // 2):
                            cls = _cls(2 * g, u)
                            sct = [scp.tile([128, 1024], f32, tag="sc", name=f"sct{i}") for i in range(2)]
                            for ti in range(2):
                                t = 2 * g + ti
                                for ah in range(2):
                                    nc.tensor.matmul(
                                        sct[ah][:, ti * 512:(ti + 1) * 512],
                                        lhsT=kt_sb[ah * 64:(ah + 1) * 64, hp,
                                                   t * 128:(t + 1) * 128],
                                        rhs=qt_sb[ah * 64:(ah + 1) * 64, hp,
                                                  u * 512:(u + 1) * 512],
                                        start=True, stop=True,
                                        tile_position=(ah * 64, 0),
                                    )
                            for ah in range(2):
                                lh = 2 * hp + ah
                                pt = ppool.tile([128, 1024], f16, tag="pt", bufs=6)
                                nc.scalar.activation(
                                    pt[:], sct[ah][:], AF.Exp,
                                    bias=cb_sb[:, lh, cls:cls + 1], scale=1.0,
                                )
                                if cls == 0 and _lvl >= 2:
                                    src = ppool.tile([128, 1024], f16, tag="pt2", bufs=6)
                                    for ti in range(2):
                                        nc.vector.tensor_mul(
                                            src[:, ti * 512:(ti + 1) * 512],
                                            pt[:, ti * 512:(ti + 1) * 512],
                                            eb_sb[:, lh, _didx(2 * g + ti, u), :],
                                        )
                                else:
                                    src = pt
                                if _lvl >= 3:
                                    for ti in range(2):
                                        t = 2 * g + ti
                                        vsl = v_sb[:, t, :].rearrange(
                                            "p (h x) -> p h x", x=65)[:, ah + 2 * hp, :]
                                        nav[ah] += 1
                                        nc.tensor.matmul(
                                            ctxp[ah][:],
                                            lhsT=vsl,
                                            rhs=src[:, ti * 512:(ti + 1) * 512],
                                            start=(nav[ah] == 1), stop=(nav[ah] == NKT),
                                        )
                                elif g == 0 and ah == 0:
                                    dbg = opool.tile([128, 512], f32, tag="ob", name="dbg")
                                    nc.vector.tensor_copy(dbg[:], src[:, 0:512])
                                    nc.sync.dma_start(
                                        outd[(u * 2 + hp) * 128:
                                             (u * 2 + hp + 1) * 128, :], dbg[:])
                        if _lvl < 3:
                            continue
                        # normalization for both heads of this pair
                        for ah in range(2):
                            ctxf = cpool.tile([65, 512], f32, tag="ctxf", bufs=3)
                            nc.vector.tensor_copy(ctxf[:], ctxp[ah][:])
                            if _lvl < 4:
                                nc.sync.dma_start(
                                    outd[(u * 2 + hp) * 128 + ah * 64:
                                         (u * 2 + hp) * 128 + ah * 64 + 65, :],
                                    ctxf[:],
                                )
                                continue
                            lp0 = cpool.tile([1, 512], f32, tag="lp0")
                            nc.sync.dma_start(lp0[:], ctxf[64:65, :])
                            linv = cpool.tile([1, 512], f32, tag="linv")
                            nc.vector.reciprocal_approx_fast(linv[:], lp0[:])
                            linvb = cpool.tile([1, 512], mybir.dt.float16, tag="linvb")
                            nc.vector.tensor_scalar_mul(linvb[:], linv[:], 256.0)
                            bc = c1p.tile([64, 512], f32, tag="cp", bufs=2)
                            nc.tensor.matmul(bc[:], lhsT=ones_c[:], rhs=linvb[:],
                                             start=True, stop=True)
                            cxn = cpool.tile([64, 512], f32r, tag="cx", bufs=6,
                                             name=f"cx{hp}{ah}")
                            nc.vector.tensor_mul(cxn[:], bc[:], ctxf[0:64, :])
                            cx[2 * hp + ah] = cxn
                        if _lvl == 4:
                            nc.sync.dma_start(
                                outd[(u * 2 + hp) * 128:(u * 2 + hp) * 128 + 64, :],
                                cx[2 * hp][:].bitcast(f32),
                            )
                    if _lvl < 5:
                        continue
                    # O-projection for this q-unit: accumulate all 4 heads
                    for qs in range(4):
                        po = c1p.tile([128, 512], f32, tag="cp", bufs=2)
                        for lh in range(4):
                            nc.tensor.matmul(
                                po[:],
                                lhsT=cx[lh][:, qs * 128:(qs + 1) * 128],
                                rhs=wo_sb[:, lh, :],
                                start=(lh == 0), stop=(lh == 3),
                            )
                        ob = opool.tile([128, 512], f32, tag="ob")
                        nc.vector.tensor_copy(ob[:], po[:])
                        nc.sync.dma_start(
                            outd[u * 512 + qs * 128: u * 512 + (qs + 1) * 128, :],
                            ob[:],
                        )
    nc.compile()
    return nc


_PROGRAM = None


def _get_program():
    global _PROGRAM
    if _PROGRAM is None:
        _PROGRAM = build_program()
    return _PROGRAM


# index table for the in-band Toeplitz bias blocks, shared across heads
_IDX = None


def _idx_table():
    global _IDX
    if _IDX is None:
        p = np.arange(128)[:, None]
        f = np.arange(512)[None, :]
        blocks = []
        for didx in range(8):
            delta = didx * 128 - 256
            blocks.append(np.clip(delta + p - f + 255, 0, 510))
        _IDX = np.stack(blocks, axis=0)  # [8, 128, 512]
    return _IDX


def kernel(**inputs):
    import ml_dtypes

    query = np.asarray(inputs["query"], dtype=np.float32)
    key = np.asarray(inputs["key"], dtype=np.float32)
    value = np.asarray(inputs["value"], dtype=np.float32)
    mask = np.asarray(inputs["mask"])
    Wq = np.asarray(inputs["Wq"], dtype=np.float32)
    Wk = np.asarray(inputs["Wk"], dtype=np.float32)
    Wv = np.asarray(inputs["Wv"], dtype=np.float32)
    Wo = np.asarray(inputs["Wo"], dtype=np.float32)
    bo = np.asarray(inputs["bo"], dtype=np.float32)
    rel_bias = np.asarray(inputs["rel_bias"], dtype=np.float32)

    if not np.all(mask != 0):
        raise NotImplementedError("kernel assumes an all-ones attention mask")

    nc = _get_program()
    idx = _idx_table()
    scale = np.float32(1.0 / np.sqrt(DK))

    in_maps = []
    for c in range(NCORES):
        b = c // 2
        hbase = (c % 2) * 4
        rows = slice(hbase * 64, (hbase + 4) * 64)

        wq_arr = np.ascontiguousarray(
            (Wq[rows, :] * scale).T.reshape(4, 128, 256).swapaxes(0, 1))
        wk_arr = np.ascontiguousarray(
            Wk[rows, :].T.reshape(4, 128, 256).swapaxes(0, 1))
        wv_arr = np.ascontiguousarray(
            Wv[rows, :].T.reshape(4, 128, 256).swapaxes(0, 1))

        wo_arr = np.empty((64, 4, 512), dtype=np.float32)
        eb_arr = np.empty((128, 4, 8, 512), dtype=np.float16)
        cb_arr = np.zeros((128, 4, 3), dtype=np.float32)
        for lh in range(4):
            g = hbase + lh
            wo_arr[:, lh, :] = Wo[:, g * 64:(g + 1) * 64].T * (1.0 / 256.0)
            tbl = rel_bias[g]
            eb_arr[:, lh, :, :] = np.exp(tbl)[idx].transpose(1, 0, 2)
            cb_arr[:, lh, 1] = tbl[0]
            cb_arr[:, lh, 2] = tbl[510]

        bf = np.float16
        in_maps.append({
            "xqT": np.ascontiguousarray(query[b].T).astype(bf),
            "xkT": np.ascontiguousarray(key[b].T).astype(bf),
            "xvT": np.ascontiguousarray(value[b].T).astype(bf),
            "wq": wq_arr.astype(bf), "wk": wk_arr.astype(bf),
            "wv": wv_arr.astype(bf), "wo": wo_arr,
            "eb": eb_arr, "cb": cb_arr,
        })

    res = run_bass_kernel_spmd(nc, in_maps, list(range(NCORES)), trace=False)

    out = np.zeros((B, S, D), dtype=np.float32)
    for c in range(NCORES):
        out[c // 2] += res.results[c]["out"]
    out += bo[None, None, :]
    return out


# revision 23
# speedup vs baseline: 1.0366x; 1.0366x over previous
"""Trainium2 Bass kernel for nn_MultiHeadAttention_34162169872901.

MultiHeadAttention (B=4, S=2048, d_model=512, 8 heads, d_k=64) with a
relative-position bias table (511 entries, clamp +-255) and an all-ones mask.

Sharding (8 NeuronCores): core c handles batch b = c//2 and 4 of the 8 heads
(c%2 selects the head half) -- data parallel on B, tensor parallel on heads.
Each core computes its 4 heads' Q/K/V projections, the full attention for its
batch, normalization, and its partial output projection; the host sums the two
partial outputs per batch (and adds the output bias bo).

On-device layout / algorithm per core:
  - Host pre-transposes activations to [d_model, S] so the contraction dim is
    on partitions, and pre-arranges weight slices; the 1/sqrt(d_k) scale is
    folded into Wq.
  - Projections produce QT/KT in [head-pair(128), S] layout and V in natural
    [S, d] layout (with a ones column per head for the softmax denominator).
  - Scores are computed transposed (S^T[k, q], k on partitions) so the
    relative-position bias becomes per-(k-tile, q-chunk) Toeplitz blocks;
    blocks fully outside the clamp band are constants folded into the exp's
    per-partition bias; in-band blocks multiply exp(scores) by precomputed
    exp(bias) blocks in bf16 on the vector engine.
  - AV matmul uses V (plus a ones column) as the stationary operand, giving
    ctx^T[d, q] and the softmax denominator l[q] in one accumulation chain.
  - Normalization: approximate reciprocal of l, broadcast to 64 rows via a
    tiny matmul, fused multiply into the O-projection stationary.
  - O-projection accumulates all 4 local heads into [q, 512] PSUM tiles.
"""

import sys
import types

import numpy as np

B = 4
S = 2048
D = 512
NHEAD = 8
DK = 64
NCORES = 8
MAX_REL = 255
NKT = S // 128   # 16 k-tiles
NU = S // 512    # 4 q-units


def _install_axon_hooks():
    """Provide antenv.axon_hooks (missing in this image) so bass_utils'
    trace path can be used; harmless when tracing is off."""
    try:
        import antenv
    except ImportError:
        return
    try:
        from antenv.axon_hooks import get_axon_ntff_profile_hook  # noqa: F401
        return
    except ImportError:
        pass
    hook = None
    try:
        from trn_agent_boot.trn_boot import _ntff_profile_via_ctypes
        hook = _ntff_profile_via_ctypes("/opt/axon/libaxon_pjrt.so")
    except Exception:
        hook = None
    m = types.ModuleType("antenv.axon_hooks")
    m.get_axon_ntff_profile_hook = lambda: hook
    m.set_axon_ntff_profile_hook = lambda h: None
    sys.modules["antenv.axon_hooks"] = m
    antenv.axon_hooks = m


_install_axon_hooks()

import concourse.bass as bass  # noqa: E402
import concourse.bacc as bacc  # noqa: E402
import concourse.mybir as mybir  # noqa: E402
from concourse import tile  # noqa: E402
from concourse.bass_utils import run_bass_kernel_spmd  # noqa: E402
from concourse.vector_clock import ScopedClock as _ScopedClock  # noqa: E402

f32 = mybir.dt.float32
f32r = mybir.dt.float32r
bf16 = mybir.dt.bfloat16
f16 = mybir.dt.float16
AF = mybir.ActivationFunctionType


def _patched_drain_and_barrier(self, tick_clock, wait_clock):
    # walrus in this container rejects >2 sem waits on one instruction; emit
    # the tail-drain waits as standalone wait instructions instead.
    nc = self.nc
    dummy = mybir.InstNoOp(name="drain-wait-probe", engine=mybir.EngineType.SP)
    wait_clock.add_sem_waits(dummy, _ScopedClock({None: tick_clock.global_clock}))
    handles = {h.name: h for h in self.sems.allocated().values()}
    si = dummy.sync_info
    for w in (si.on_wait if si is not None else []):
        nc.sync.wait_ge(handles[w.ant_name], w.wait_value)
    nc.sync.drain()
    nc.all_engine_barrier()
    popped = nc._tile_sem_poison_stack.pop()
    assert popped is self._sem_poison
    nc.clear_and_free_semaphores(list(self.sems.allocated().values()))
    nc.all_engine_barrier()


tile.TileContext._drain_and_barrier = _patched_drain_and_barrier


def _delta(t, u):
    # key-tile offset minus query-chunk offset; bias entry index is
    # delta + (p - f) + 255 clipped to [0, 510]
    return 128 * t - 512 * u


def _cls(t, u):
    d = _delta(t, u)
    if d <= -384:
        return 1  # whole block clamps to table[0]
    if d >= 768:
        return 2  # whole block clamps to table[510]
    return 0      # in-band: needs the Toeplitz block


def _didx(t, u):
    return (_delta(t, u) + 256) // 128  # 0..7 for in-band blocks


def build_program():
    nc = bacc.Bacc()

    xqT = nc.declare_dram_parameter("xqT", [D, S], f16, isOutput=False)
    xkT = nc.declare_dram_parameter("xkT", [D, S], f16, isOutput=False)
    xvT = nc.declare_dram_parameter("xvT", [D, S], f16, isOutput=False)
    wq = nc.declare_dram_parameter("wq", [128, 4, 256], f16, isOutput=False)
    wk = nc.declare_dram_parameter("wk", [128, 4, 256], f16, isOutput=False)
    wv = nc.declare_dram_parameter("wv", [128, 4, 256], f16, isOutput=False)
    wo = nc.declare_dram_parameter("wo", [64, 4, 512], f32r, isOutput=False)
    ebd = nc.declare_dram_parameter("eb", [128, 4, 8, 512], f16, isOutput=False)
    cbd = nc.declare_dram_parameter("cb", [128, 4, 3], f32, isOutput=False)
    outd = nc.declare_dram_parameter("out", [S, D], f32, isOutput=True)

    with tile.TileContext(nc) as tc:
        with (
            tc.tile_pool(name="sb", bufs=1) as pool,
            tc.tile_pool(name="xt", bufs=2) as xpool,
            tc.tile_pool(name="pt", bufs=4) as ppool,
            tc.tile_pool(name="cxp", bufs=3) as cpool,
            tc.tile_pool(name="obp", bufs=4) as opool,
        ):
            # ---- persistent SBUF tiles -------------------------------------
            wq_sb = pool.tile([128, 4, 256], f16, tag="wq")
            wk_sb = pool.tile([128, 4, 256], f16, tag="wk")
            wv_sb = pool.tile([128, 4, 256], f16, tag="wv")
            wo_sb = pool.tile([64, 4, 512], f32r, tag="wo")
            eb_sb = pool.tile([128, 4, 8, 512], f16, tag="eb")
            cb_sb = pool.tile([128, 4, 3], f32, tag="cb")
            qt_sb = pool.tile([128, 2, S], f16, tag="qt")
            kt_sb = pool.tile([128, 2, S], f16, tag="kt")
            v_sb = pool.tile([128, NKT, 4 * 65], f16, tag="v")
            ones_c = pool.tile([1, 64], mybir.dt.float16, tag="ones")
            warm = pool.tile([128, 16], f32, tag="warm")

            nc.sync.dma_start(wq_sb[:], wq[:])
            nc.sync.dma_start(wk_sb[:], wk[:])
            nc.sync.dma_start(wv_sb[:], wv[:])
            nc.vector.memset(ones_c[:], 1.0)
            # preload the exp table while DMAs stream in
            nc.vector.memset(warm[:], 0.0)
            nc.scalar.activation(warm[:], warm[:], AF.Exp, bias=0.0, scale=1.0)

            # ---- phase A: projections --------------------------------------
            with tc.tile_pool(name="pa", bufs=8, space="PSUM") as pa:
                _dma_rest = True
                # Q and K -> [head-pair 128, S] (transposed) layout
                # xv streams on the gpsimd queue ahead of eb/cb/wo
                xts = []
                for ct in range(4):
                    xv_t = xpool.tile([128, S], f16, tag="xv", bufs=4, name=f"xv{ct}")
                    nc.gpsimd.dma_start(xv_t[:], xvT[ct * 128:(ct + 1) * 128, :])
                    xts.append(xv_t)
                nc.gpsimd.dma_start(eb_sb[:], ebd[:])
                nc.gpsimd.dma_start(cb_sb[:], cbd[:])
                nc.gpsimd.dma_start(wo_sb[:], wo[:])
                # K and Q hp0 stream per-arriving c-tile (8 banks); hp1 and V
                # run as dense backlog to keep the PE warm into phase B
                kts, qts = [], []
                for ct in range(4):
                    xk_t = xpool.tile([128, S], f16, tag="xt", name=f"xk{ct}")
                    nc.sync.dma_start(xk_t[:], xkT[ct * 128:(ct + 1) * 128, :])
                    kts.append(xk_t)
                    xq_t = xpool.tile([128, S], f16, tag="xq", bufs=4, name=f"xq{ct}")
                    nc.scalar.dma_start(xq_t[:], xqT[ct * 128:(ct + 1) * 128, :])
                    qts.append(xq_t)
                for hp in range(2):
                    pks = {sc: pa.tile([128, 512], f32, tag="pa", name=f"pk{hp}_{sc}")
                           for sc in range(4)}
                    pqs = {sc: pa.tile([128, 512], f32, tag="pa", name=f"pq{hp}_{sc}")
                           for sc in range(4)}
                    for ct in range(4):
                        for sc in range(4):
                            nc.tensor.matmul(
                                pks[sc][:],
                                lhsT=wk_sb[:, ct, hp * 128:(hp + 1) * 128],
                                rhs=kts[ct][:, sc * 512:(sc + 1) * 512],
                                start=(ct == 0), stop=(ct == 3),
                            )
                            nc.tensor.matmul(
                                pqs[sc][:],
                                lhsT=wq_sb[:, ct, hp * 128:(hp + 1) * 128],
                                rhs=qts[ct][:, sc * 512:(sc + 1) * 512],
                                start=(ct == 0), stop=(ct == 3),
                            )
                    for sc in range(4):
                        nc.vector.tensor_copy(
                            kt_sb[:, hp, sc * 512:(sc + 1) * 512], pks[sc][:])
                        nc.vector.tensor_copy(
                            qt_sb[:, hp, sc * 512:(sc + 1) * 512], pqs[sc][:])
                # V -> natural [s, d] layout in bf16, one 65-col group per head
                # one accumulation group open per bank at a time: run each
                # s-tile's 4-step contraction to completion (xv tiles resident)
                pv = [pa.tile([128, 512], f32, tag="pa", name=f"pv{i}") for i in range(8)]
                for st in range(NKT):
                    for ct in range(4):
                        nc.tensor.matmul(
                            pv[st // 2][:, (st % 2) * 256:(st % 2) * 256 + 256],
                            lhsT=xts[ct][:, st * 128:(st + 1) * 128],
                            rhs=wv_sb[:, ct, :],
                            start=(ct == 0), stop=(ct == 3),
                        )
                for st in range(NKT):
                    vslice = v_sb[:, st, :].rearrange("p (h x) -> p h x", x=65)
                    nc.vector.tensor_copy(
                        vslice[:, :, 0:64],
                        pv[st // 2][:, (st % 2) * 256:(st % 2) * 256 + 256]
                        .rearrange("p (h x) -> p h x", x=64),
                    )
                    nc.vector.memset(vslice[:, :, 64:65], 1.0)

            # ---- phase B: attention + normalization + O-projection --------
            import os as _os
            _phase = _os.environ.get("KPHASE", "full")
            if _phase == "A":
                ob0 = opool.tile([128, 512], f32, tag="ob", name="ob0")
                nc.vector.tensor_copy(ob0[:], qt_sb[:, 0, 0:512].bitcast(f32))
                nc.sync.dma_start(outd[0:128, :], ob0[:])
                ob1 = opool.tile([128, 256], f32, tag="ob1", name="ob1")
                nc.vector.tensor_copy(ob1[:], v_sb[:, 0, 0:256])
                nc.sync.dma_start(outd[128:256, 0:256], ob1[:])
            _enable_b = _phase != "A"
            with (
                tc.tile_pool(name="sc", bufs=3, space="PSUM") as scp,
                tc.tile_pool(name="c1", bufs=1, space="PSUM") as c1p,
            ):
                _lvl = int(_os.environ.get("KLEVEL", "5"))
                for u in (range(NU) if _enable_b else []):
                    cx = {}
                    for hp in range(2):
                        if _lvl >= 3:
                            ctxp = [c1p.tile([65, 512], f32, tag="cp", bufs=2, name=f"ctxp{i}") for i in range(2)]
                        nav = [0, 0]
                        gorder = sorted(range(NKT // 2),
                                        key=lambda g: (_cls(2 * g, u) == 0, g))
                        for g in gorder:
                            cls = _cls(2 * g, u)
                            sct = [scp.tile([128, 1024], f32, tag="sc", name=f"sct{i}") for i in range(2)]
                            for ti in range(2):
                                t = 2 * g + ti
                                for ah in range(2):
                                    nc.tensor.matmul(
                                        sct[ah][:, ti * 512:(ti + 1) * 512],
                                        lhsT=kt_sb[ah * 64:(ah + 1) * 64, hp,
                                                   t * 128:(t + 1) * 128],
                                        rhs=qt_sb[ah * 64:(ah + 1) * 64, hp,
                                                  u * 512:(u + 1) * 512],
                                        start=True, stop=True,
                                        tile_position=(ah * 64, 0),
                                    )
                            for ah in range(2):
                                lh = 2 * hp + ah
                                pt = ppool.tile([128, 1024], f16, tag="pt", bufs=6)
                                nc.scalar.activation(
                                    pt[:], sct[ah][:], AF.Exp,
                                    bias=cb_sb[:, lh, cls:cls + 1], scale=1.0,
                                )
                                if cls == 0 and _lvl >= 2:
                                    src = ppool.tile([128, 1024], f16, tag="pt2", bufs=6)
                                    for ti in range(2):
                                        nc.vector.tensor_mul(
                                            src[:, ti * 512:(ti + 1) * 512],
                                            pt[:, ti * 512:(ti + 1) * 512],
                                            eb_sb[:, lh, _didx(2 * g + ti, u), :],
                                        )
                                else:
                                    src = pt
                                if _lvl >= 3:
                                    for ti in range(2):
                                        t = 2 * g + ti
                                        vsl = v_sb[:, t, :].rearrange(
                                            "p (h x) -> p h x", x=65)[:, ah + 2 * hp, :]
                                        nav[ah] += 1
                                        nc.tensor.matmul(
                                            ctxp[ah][:],
                                            lhsT=vsl,
                                            rhs=src[:, ti * 512:(ti + 1) * 512],
                                            start=(nav[ah] == 1), stop=(nav[ah] == NKT),
                                        )
                                elif g == 0 and ah == 0:
                                    dbg = opool.tile([128, 512], f32, tag="ob", name="dbg")
                                    nc.vector.tensor_copy(dbg[:], src[:, 0:512])
                                    nc.sync.dma_start(
                                        outd[(u * 2 + hp) * 128:
                                             (u * 2 + hp + 1) * 128, :], dbg[:])
                        if _lvl < 3:
                            continue
                        # normalization for both heads of this pair
                        for ah in range(2):
                            ctxf = cpool.tile([65, 512], f32, tag="ctxf", bufs=3)
                            nc.vector.tensor_copy(ctxf[:], ctxp[ah][:])
                            if _lvl < 4:
                                nc.sync.dma_start(
                                    outd[(u * 2 + hp) * 128 + ah * 64:
                                         (u * 2 + hp) * 128 + ah * 64 + 65, :],
                                    ctxf[:],
                                )
                                continue
                            lp0 = cpool.tile([1, 512], f32, tag="lp0")
                            nc.sync.dma_start(lp0[:], ctxf[64:65, :])
                            linv = cpool.tile([1, 512], f32, tag="linv")
                            nc.vector.reciprocal_approx_fast(linv[:], lp0[:])
                            linvb = cpool.tile([1, 512], mybir.dt.float16, tag="linvb")
                            nc.vector.tensor_scalar_mul(linvb[:], linv[:], 256.0)
                            bc = c1p.tile([64, 512], f32, tag="cp", bufs=2)
                            nc.tensor.matmul(bc[:], lhsT=ones_c[:], rhs=linvb[:],
                                             start=True, stop=True)
                            cxn = cpool.tile([64, 512], f32r, tag="cx", bufs=6,
                                             name=f"cx{hp}{ah}")
                            nc.vector.tensor_mul(cxn[:], bc[:], ctxf[0:64, :])
                            cx[2 * hp + ah] = cxn
                        if _lvl == 4:
                            nc.sync.dma_start(
                                outd[(u * 2 + hp) * 128:(u * 2 + hp) * 128 + 64, :],
                                cx[2 * hp][:].bitcast(f32),
                            )
                    if _lvl < 5:
                        continue
                    # O-projection for this q-unit: accumulate all 4 heads
                    for qs in range(4):
                        po = c1p.tile([128, 512], f32, tag="cp", bufs=2)
                        for lh in range(4):
                            nc.tensor.matmul(
                                po[:],
                                lhsT=cx[lh][:, qs * 128:(qs + 1) * 128],
                                rhs=wo_sb[:, lh, :],
                                start=(lh == 0), stop=(lh == 3),
                            )
                        ob = opool.tile([128, 512], f32, tag="ob")
                        nc.vector.tensor_copy(ob[:], po[:])
                        nc.sync.dma_start(
                            outd[u * 512 + qs * 128: u * 512 + (qs + 1) * 128, :],
                            ob[:],
                        )
    nc.compile()
    return nc


_PROGRAM = None


def _get_program():
    global _PROGRAM
    if _PROGRAM is None:
        _PROGRAM = build_program()
    return _PROGRAM


# index table for the in-band Toeplitz bias blocks, shared across heads
_IDX = None


def _idx_table():
    global _IDX
    if _IDX is None:
        p = np.arange(128)[:, None]
        f = np.arange(512)[None, :]
        blocks = []
        for didx in range(8):
            delta = didx * 128 - 256
            blocks.append(np.clip(delta + p - f + 255, 0, 510))
        _IDX = np.stack(blocks, axis=0)  # [8, 128, 512]
    return _IDX


def kernel(**inputs):
    import ml_dtypes

    query = np.asarray(inputs["query"], dtype=np.float32)
    key = np.asarray(inputs["key"], dtype=np.float32)
    value = np.asarray(inputs["value"], dtype=np.float32)
    mask = np.asarray(inputs["mask"])
    Wq = np.asarray(inputs["Wq"], dtype=np.float32)
    Wk = np.asarray(inputs["Wk"], dtype=np.float32)
    Wv = np.asarray(inputs["Wv"], dtype=np.float32)
    Wo = np.asarray(inputs["Wo"], dtype=np.float32)
    bo = np.asarray(inputs["bo"], dtype=np.float32)
    rel_bias = np.asarray(inputs["rel_bias"], dtype=np.float32)

    if not np.all(mask != 0):
        raise NotImplementedError("kernel assumes an all-ones attention mask")

    nc = _get_program()
    idx = _idx_table()
    scale = np.float32(1.0 / np.sqrt(DK))

    in_maps = []
    for c in range(NCORES):
        b = c // 2
        hbase = (c % 2) * 4
        rows = slice(hbase * 64, (hbase + 4) * 64)

        wq_arr = np.ascontiguousarray(
            (Wq[rows, :] * scale).T.reshape(4, 128, 256).swapaxes(0, 1))
        wk_arr = np.ascontiguousarray(
            Wk[rows, :].T.reshape(4, 128, 256).swapaxes(0, 1))
        wv_arr = np.ascontiguousarray(
            Wv[rows, :].T.reshape(4, 128, 256).swapaxes(0, 1))

        wo_arr = np.empty((64, 4, 512), dtype=np.float32)
        eb_arr = np.empty((128, 4, 8, 512), dtype=np.float16)
        cb_arr = np.zeros((128, 4, 3), dtype=np.float32)
        for lh in range(4):
            g = hbase + lh
            wo_arr[:, lh, :] = Wo[:, g * 64:(g + 1) * 64].T * (1.0 / 256.0)
            tbl = rel_bias[g]
            eb_arr[:, lh, :, :] = np.exp(tbl)[idx].transpose(1, 0, 2)
            cb_arr[:, lh, 1] = tbl[0]
            cb_arr[:, lh, 2] = tbl[510]

        bf = np.float16
        in_maps.append({
            "xqT": np.ascontiguousarray(query[b].T).astype(bf),
            "xkT": np.ascontiguousarray(key[b].T).astype(bf),
            "xvT": np.ascontiguousarray(value[b].T).astype(bf),
            "wq": wq_arr.astype(bf), "wk": wk_arr.astype(bf),
            "wv": wv_arr.astype(bf), "wo": wo_arr,
            "eb": eb_arr, "cb": cb_arr,
        })

    res = run_bass_kernel_spmd(nc, in_maps, list(range(NCORES)), trace=False)

    out = np.zeros((B, S, D), dtype=np.float32)
    for c in range(NCORES):
        out[c // 2] += res.results[c]["out"]
    out += bo[None, None, :]
    return out
